# revision 3
# baseline (speedup 1.0000x reference)
"""Graphwise KL loss (segment_reduce) on 8 trn2 NeuronCores.

Strategy (v3, bf16 + lnq input + 4-engine balance + full prefetch):
  Host:
    - Cast y_true/weight to bf16; precompute lnq = bf16(ln(y_pred + 1e-8))
      (y_pred only ever appears through ln(max(y_pred, eps))).
    - Pre-transpose each [TILE_F, 128] chunk so an SBUF column holds 128
      CONSECUTIVE elements -> PE can do the block sums across partitions.
  Device (per core, 4 tiles of [128, 2048] bf16, all inputs prefetched):
    sync  : 12 HWDGE input loads issued immediately + final 64KB store
    ACT   : warmup (Ln table load) ; lp(t) = Ln(pr + 1e-37)
    DVE   : pr(t) = yt*w ; e1(t) = pr*d   (bf16 2x mode) ; final psum copy
    GPSIMD: d(t) = lp - lnq
    PE    : per 128-col chunk: matmul(stationary=chunk, moving=ones[128,1])
            -> psum[:, col] = 128-element block sums of e1 / pr
  Host assembly (fp64): BLK=128 block sums; boundary partials recomputed
  from the bf16-cast inputs; with S_g = max(B_g, EPS):
      total = mean_g (A_g - B_g*ln(S_g)) / S_g.

  Raw Bass: standalone wait_ge only; per-tile DMA sems (+48 = 3 loads x 16).
"""

import numpy as np
from ml_dtypes import bfloat16

N_TOTAL = 8388608
N_CORES = 8
N_LOCAL = N_TOTAL // N_CORES      # 1048576
P = 128
TILE_F = 2048
TILE_ELEMS = P * TILE_F           # 262144
N_TILES = N_LOCAL // TILE_ELEMS   # 4
BLK = 128
CHUNK = 128
CPT = TILE_F // CHUNK             # 16
COLS = N_TILES * CPT              # 64
N_BLOCKS_LOCAL = N_LOCAL // BLK   # 8192
EPS = 1e-8
TINY = 1e-37

_CACHE = {}


def _check_one_wait(nc):
    bad = []
    for f in nc.m.functions:
        for bb in f.blocks:
            for inst in bb.instructions:
                si = inst.sync_info
                if si and si.on_wait and len(si.on_wait) > 1:
                    if "EventSem" not in type(inst).__name__:
                        bad.append((type(inst).__name__, inst.name, len(si.on_wait)))
    assert not bad, f"multi-wait instructions remain: {bad}"


def _build_program():
    import concourse.bass as bass
    import concourse.mybir as mybir

    f32 = mybir.dt.float32
    bf16 = mybir.dt.bfloat16
    Ln = mybir.ActivationFunctionType.Ln

    nc = bass.Bass()

    ct = nc.alloc_sbuf_tensor(f"const-f32-tiny", [128, 1], f32)
    nc.gpsimd.memset(ct.ap(), TINY)
    nc.const_aps.aps[(f32, TINY)] = ct.ap()
    nc.all_engine_barrier()

    yt = nc.declare_dram_parameter("yt", [N_LOCAL], bf16, isOutput=False)
    w = nc.declare_dram_parameter("w", [N_LOCAL], bf16, isOutput=False)
    q = nc.declare_dram_parameter("q", [N_LOCAL], bf16, isOutput=False)  # lnq
    o = nc.declare_dram_parameter("o", [P * 2 * COLS], f32, isOutput=True)

    yt3 = yt[:].rearrange("(t p f) -> t p f", p=P, f=TILE_F)
    w3 = w[:].rearrange("(t p f) -> t p f", p=P, f=TILE_F)
    q3 = q[:].rearrange("(t p f) -> t p f", p=P, f=TILE_F)
    o2 = o[:].rearrange("(p f) -> p f", p=P)

    def bufn(name, n, shape, dt):
        return [nc.alloc_sbuf_tensor(f"{name}{i}", shape, dt).ap() for i in range(n)]

    # full prefetch: one SBUF tile per input tile
    t_yt = bufn("t_yt", N_TILES, [P, TILE_F], bf16)
    t_w = bufn("t_w", N_TILES, [P, TILE_F], bf16)
    t_q = bufn("t_q", N_TILES, [P, TILE_F], bf16)
    # double-buffered intermediates
    t_pr = bufn("t_pr", 2, [P, TILE_F], bf16)
    t_lp = bufn("t_lp", 2, [P, TILE_F], bf16)
    t_d = bufn("t_d", 2, [P, TILE_F], bf16)
    t_e1 = bufn("t_e1", 2, [P, TILE_F], bf16)
    out_sb = nc.alloc_sbuf_tensor("out_sb", [P, 2 * COLS], f32).ap()
    warm_sb = nc.alloc_sbuf_tensor("warm_sb", [P, 1], f32).ap()

    ps = nc.alloc_psum_tensor("ps", [P, 2 * COLS], f32).ap()

    ones_bf = nc.const_aps.aps[(bf16, 1.0)]
    zero_f32 = nc.const_aps.aps[(f32, 0.0)]

    s_t = [nc.alloc_semaphore(f"s_t{i}") for i in range(N_TILES)]  # +48/tile
    s_act = nc.alloc_semaphore("s_act")   # warm=1, lp(t)=t+2
    s_dve = nc.alloc_semaphore("s_dve")
    s_gp = nc.alloc_semaphore("s_gp")     # d(t) -> t+1
    s_pe = nc.alloc_semaphore("s_pe")     # tile t matmuls done -> t+1
    s_out = nc.alloc_semaphore("s_out")

    # DVE order: pr0 pr1 e1_0 pr2 e1_1 pr3 e1_2 e1_3 copy
    dve_idx = {}
    n = 0
    order = [("pr", 0), ("pr", 1)]
    for t in range(N_TILES):
        order.append(("e1", t))
        if t + 2 < N_TILES:
            order.append(("pr", t + 2))
    for kind, t in order:
        n += 1
        dve_idx[(kind, t)] = n
    n_dve_total = n + 1  # + final psum copy

    with nc.Block(no_gpsimd_drain=True) as block:

        @block.sync
        def _(s):
            for t in range(N_TILES):
                s.dma_start(t_yt[t], yt3[t, :, :]).then_inc(s_t[t], 16)
                s.dma_start(t_w[t], w3[t, :, :]).then_inc(s_t[t], 16)
                s.dma_start(t_q[t], q3[t, :, :]).then_inc(s_t[t], 16)
            s.wait_ge(s_dve, n_dve_total)
            s.dma_start(o2, out_sb).then_inc(s_out, 16)
            s.wait_ge(s_out, 16)

        @block.scalar
        def _(a):
            a.activation(warm_sb, zero_f32, Ln, bias=TINY).then_inc(s_act, 1)
            for t in range(N_TILES):
                buf = t % 2
                a.wait_ge(s_dve, dve_idx[("pr", t)])
                if t >= 2:
                    # lp[buf] was read by gpsimd d(t-2)
                    a.wait_ge(s_gp, t - 1)
                a.activation(t_lp[buf], t_pr[buf], Ln, bias=TINY).then_inc(s_act, 1)

        @block.gpsimd
        def _(g):
            for t in range(N_TILES):
                buf = t % 2
                g.wait_ge(s_act, t + 2)      # lp(t) done
                g.wait_ge(s_t[t], 48)        # lnq(t) loaded
                if t >= 2:
                    # d[buf] was read by DVE e1(t-2)
                    g.wait_ge(s_dve, dve_idx[("e1", t - 2)])
                g.tensor_sub(t_d[buf], t_lp[buf], t_q[t]).then_inc(s_gp, 1)

        @block.vector
        def _(v):
            def emit_pr(t):
                buf = t % 2
                v.wait_ge(s_t[t], 48)
                if t >= 2:
                    # pr/e1[buf] were consumed by PE matmuls of tile t-2
                    v.wait_ge(s_pe, t - 1)
                v.tensor_mul(t_pr[buf], t_yt[t], t_w[t]).then_inc(s_dve, 1)

            def emit_e1(t):
                buf = t % 2
                v.wait_ge(s_gp, t + 1)       # d(t) done
                v.tensor_mul(t_e1[buf], t_pr[buf], t_d[buf]).then_inc(s_dve, 1)

            for kind, t in order:
                (emit_pr if kind == "pr" else emit_e1)(t)
            v.wait_ge(s_pe, N_TILES)
            v.tensor_copy(out_sb, ps).then_inc(s_dve, 1)

        @block.tensor
        def _(te):
            for t in range(N_TILES):
                buf = t % 2
                te.wait_ge(s_dve, dve_idx[("e1", t)])
                for c in range(CPT):
                    col = t * CPT + c
                    sl = slice(c * CHUNK, (c + 1) * CHUNK)
                    te.matmul(ps[:, col:col + 1], t_e1[buf][:, sl], ones_bf,
                              start=True, stop=True)
                    mm = te.matmul(ps[:, COLS + col:COLS + col + 1],
                                   t_pr[buf][:, sl], ones_bf,
                                   start=True, stop=True)
                mm.then_inc(s_pe, 1)

    _check_one_wait(nc)
    return nc


def _get_program():
    if "nc" not in _CACHE:
        _CACHE["nc"] = _build_program()
    return _CACHE["nc"]


def _shard(xb):
    """bf16 [N_TOTAL] -> per-core arrays in the transposed tile layout:
    dram[t, p, f] = x[core_base + t*TILE_ELEMS + f*P + p]."""
    xt = xb.reshape(N_CORES, N_TILES, TILE_F, P).transpose(0, 1, 3, 2)
    return [np.ascontiguousarray(xt[k]).reshape(N_LOCAL) for k in range(N_CORES)]


def _run_device(yt_s, w_s, q_s, trace=False):
    from concourse.bass_utils import run_bass_kernel_spmd

    nc = _get_program()
    in_maps = [
        {"yt": yt_s[k], "w": w_s[k], "q": q_s[k]} for k in range(N_CORES)
    ]
    res = run_bass_kernel_spmd(nc, in_maps, list(range(N_CORES)), trace=trace)
    bs1 = []
    bs2 = []
    for r in res.results:
        O = np.asarray(r["o"]).reshape(P, 2 * COLS)
        bs1.append(O[:, :COLS].T.ravel())   # block b = 128*col + m
        bs2.append(O[:, COLS:].T.ravel())
    return np.concatenate(bs1), np.concatenate(bs2), res


def kernel(y_pred, y_true, weight, segment_ptr, _trace=False):
    ptr = np.asarray(segment_ptr).astype(np.int64).reshape(-1)
    n = N_TOTAL
    G = ptr.shape[0] - 1

    yp = np.ascontiguousarray(np.asarray(y_pred), dtype=np.float32)
    yt_b = np.ascontiguousarray(np.asarray(y_true), dtype=np.float32).astype(bfloat16)
    w_b = np.ascontiguousarray(np.asarray(weight), dtype=np.float32).astype(bfloat16)
    lnq_b = np.log(yp.astype(np.float64) + EPS).astype(np.float32).astype(bfloat16)

    bs1, bs2, res = _run_device(
        _shard(yt_b), _shard(w_b), _shard(lnq_b), trace=_trace)
    _CACHE["last_res"] = res

    # ---- host assembly in fp64 ----
    pre1 = np.empty(bs1.shape[0] + 1)
    pre1[0] = 0.0
    np.cumsum(bs1, dtype=np.float64, out=pre1[1:])
    pre2 = np.empty(bs2.shape[0] + 1)
    pre2[0] = 0.0
    np.cumsum(bs2, dtype=np.float64, out=pre2[1:])

    ptrc = np.clip(ptr, 0, n)
    b_idx = ptrc // BLK
    r = ptrc - b_idx * BLK
    seg_off = np.concatenate([[0], np.cumsum(r)])
    tot = int(seg_off[-1])
    part1 = np.zeros(ptrc.shape[0])
    part2 = np.zeros(ptrc.shape[0])
    if tot > 0:
        idx = np.repeat(ptrc - r, r) + (np.arange(tot) - np.repeat(seg_off[:-1], r))
        pr_h = (yt_b[idx].astype(np.float64) * w_b[idx].astype(np.float64))
        pr_h = pr_h.astype(bfloat16).astype(np.float64)
        e1_h = pr_h * (np.log(pr_h + TINY) - lnq_b[idx].astype(np.float64))
        nz = r > 0
        red_idx = np.minimum(seg_off[:-1][nz], tot - 1).astype(np.int64)
        part1[nz] = np.add.reduceat(e1_h, red_idx)
        part2[nz] = np.add.reduceat(pr_h, red_idx)

    C1 = pre1[b_idx] + part1
    C2 = pre2[b_idx] + part2
    A = np.diff(C1)
    Bg = np.diff(C2)
    S = np.maximum(Bg, EPS)
    total = np.sum((A - Bg * np.log(S)) / S) / max(G, 1)
    return np.float32(total)


# revision 4
# speedup vs baseline: 1.2862x; 1.2862x over previous
"""Graphwise KL loss (segment_reduce) on 8 trn2 NeuronCores.

Strategy (v4, bf16 + lnq input, DVE-only elementwise, full prefetch):
  Host:
    - Cast y_true/weight to bf16; precompute lnq = bf16(ln(y_pred + 1e-8))
      (y_pred only ever appears through ln(max(y_pred, eps))).
    - Pre-transpose each [TILE_F, 128] chunk so an SBUF column holds 128
      CONSECUTIVE elements -> PE does the block sums across partitions.
  Device (per core, 4 tiles of [128, 2048] bf16, all inputs prefetched):
    sync  : 12 HWDGE input loads (yt,w,q per tile; issued immediately,
            no waits) + final 64KB store
    ACT   : warmup (Ln table load during DMA fill) ; lp(t) = Ln(pr + 1e-37)
    DVE   : pr(t) = yt*w ; d(t) = lp - lnq ; e1(t) = pr*d (bf16 2x mode);
            final psum -> SBUF copy
    PE    : per 128-col chunk: matmul(stationary=chunk, moving=ones[128,1])
            -> psum[:, col] = 128-element block sums of e1 / pr
    GPSIMD: idle (its tensor ops contend with DVE's SBUF port)
  Host assembly (fp64): BLK=128 block sums; boundary partials recomputed
  from the bf16-cast inputs; with S_g = max(B_g, EPS):
      total = mean_g (A_g - B_g*ln(S_g)) / S_g.
"""

import numpy as np
from ml_dtypes import bfloat16

N_TOTAL = 8388608
N_CORES = 8
N_LOCAL = N_TOTAL // N_CORES      # 1048576
P = 128
TILE_F = 2048
TILE_ELEMS = P * TILE_F           # 262144
N_TILES = N_LOCAL // TILE_ELEMS   # 4
BLK = 128
CHUNK = 128
CPT = TILE_F // CHUNK             # 16
COLS = N_TILES * CPT              # 64
N_BLOCKS_LOCAL = N_LOCAL // BLK   # 8192
EPS = 1e-8
TINY = 1e-37

_CACHE = {}


def _check_one_wait(nc):
    bad = []
    for f in nc.m.functions:
        for bb in f.blocks:
            for inst in bb.instructions:
                si = inst.sync_info
                if si and si.on_wait and len(si.on_wait) > 1:
                    if "EventSem" not in type(inst).__name__:
                        bad.append((type(inst).__name__, inst.name, len(si.on_wait)))
    assert not bad, f"multi-wait instructions remain: {bad}"


def _build_program():
    import concourse.bass as bass
    import concourse.mybir as mybir

    f32 = mybir.dt.float32
    bf16 = mybir.dt.bfloat16
    Ln = mybir.ActivationFunctionType.Ln

    nc = bass.Bass()

    ct = nc.alloc_sbuf_tensor(f"const-f32-tiny", [128, 1], f32)
    mset = nc.gpsimd.memset(ct.ap(), TINY)
    nc.const_aps.aps[(f32, TINY)] = ct.ap()
    s_init = nc.alloc_semaphore("s_init")
    mset.then_inc(s_init, 1)

    yt = nc.declare_dram_parameter("yt", [N_LOCAL], bf16, isOutput=False)
    w = nc.declare_dram_parameter("w", [N_LOCAL], bf16, isOutput=False)
    q = nc.declare_dram_parameter("q", [N_LOCAL], bf16, isOutput=False)  # lnq
    o = nc.declare_dram_parameter("o", [P * 2 * COLS], f32, isOutput=True)

    yt3 = yt[:].rearrange("(t p f) -> t p f", p=P, f=TILE_F)
    w3 = w[:].rearrange("(t p f) -> t p f", p=P, f=TILE_F)
    q3 = q[:].rearrange("(t p f) -> t p f", p=P, f=TILE_F)
    o2 = o[:].rearrange("(p f) -> p f", p=P)

    def bufn(name, n, shape, dt):
        return [nc.alloc_sbuf_tensor(f"{name}{i}", shape, dt).ap() for i in range(n)]

    t_yt = bufn("t_yt", N_TILES, [P, TILE_F], bf16)
    t_w = bufn("t_w", N_TILES, [P, TILE_F], bf16)
    t_q = bufn("t_q", N_TILES, [P, TILE_F], bf16)
    t_pr = bufn("t_pr", 2, [P, TILE_F], bf16)
    t_lp = bufn("t_lp", 2, [P, TILE_F], bf16)
    t_d = bufn("t_d", 2, [P, TILE_F], bf16)
    t_e1 = bufn("t_e1", 2, [P, TILE_F], bf16)
    out_sb = nc.alloc_sbuf_tensor("out_sb", [P, 2 * COLS], f32).ap()
    warm_sb = nc.alloc_sbuf_tensor("warm_sb", [P, 1], f32).ap()

    ps = nc.alloc_psum_tensor("ps", [P, 2 * COLS], f32).ap()

    ones_bf = nc.const_aps.aps[(bf16, 1.0)]
    zero_f32 = nc.const_aps.aps[(f32, 0.0)]

    s_tw = [nc.alloc_semaphore(f"s_tw{i}") for i in range(N_TILES)]  # +32
    s_q = [nc.alloc_semaphore(f"s_q{i}") for i in range(N_TILES)]    # +16
    s_act = nc.alloc_semaphore("s_act")   # warm=1, lp(t)=t+2
    s_dve = nc.alloc_semaphore("s_dve")
    s_pe = nc.alloc_semaphore("s_pe")     # tile t matmuls done -> t+1
    s_out = nc.alloc_semaphore("s_out")

    # DVE order: pr0 pr1 [d,e1]0 pr2 [d,e1]1 pr3 [d,e1]2 [d,e1]3 copy
    dve_idx = {}
    n = 0
    order = [("pr", 0), ("pr", 1)]
    for t in range(N_TILES):
        order.append(("de", t))
        if t + 2 < N_TILES:
            order.append(("pr", t + 2))
    for kind, t in order:
        if kind == "pr":
            n += 1
            dve_idx[("pr", t)] = n
        else:
            n += 1
            dve_idx[("d", t)] = n
            n += 1
            dve_idx[("e1", t)] = n
    n_dve_total = n + 1  # + final psum copy

    with nc.Block(no_gpsimd_drain=True) as block:

        @block.sync
        def _(s):
            for t in range(N_TILES):
                s.dma_start(t_yt[t], yt3[t, :, :]).then_inc(s_tw[t], 16)
                s.dma_start(t_w[t], w3[t, :, :]).then_inc(s_tw[t], 16)
                s.dma_start(t_q[t], q3[t, :, :]).then_inc(s_q[t], 16)
            s.wait_ge(s_dve, n_dve_total)
            s.dma_start(o2, out_sb).then_inc(s_out, 16)
            s.wait_ge(s_out, 16)

        @block.scalar
        def _(a):
            a.wait_ge(s_init, 1)
            a.activation(warm_sb, zero_f32, Ln, bias=TINY).then_inc(s_act, 1)
            for t in range(N_TILES):
                buf = t % 2
                a.wait_ge(s_dve, dve_idx[("pr", t)])
                # lp[buf] reuse: was read by DVE d(t-2), which precedes pr(t)
                # in the DVE order, so the pr(t) wait covers it.
                a.activation(t_lp[buf], t_pr[buf], Ln, bias=TINY).then_inc(s_act, 1)

        @block.vector
        def _(v):
            def emit_pr(t):
                buf = t % 2
                v.wait_ge(s_tw[t], 32)
                if t >= 2:
                    # pr/e1[buf] were consumed by PE matmuls of tile t-2
                    v.wait_ge(s_pe, t - 1)
                v.tensor_mul(t_pr[buf], t_yt[t], t_w[t]).then_inc(s_dve, 1)

            def emit_de(t):
                buf = t % 2
                v.wait_ge(s_act, t + 2)   # lp(t) done
                v.wait_ge(s_q[t], 16)     # lnq(t) loaded
                v.tensor_sub(t_d[buf], t_lp[buf], t_q[t]).then_inc(s_dve, 1)
                v.wait_ge(s_dve, dve_idx[("d", t)])  # same-engine RAW
                v.tensor_mul(t_e1[buf], t_pr[buf], t_d[buf]).then_inc(s_dve, 1)

            for kind, t in order:
                (emit_pr if kind == "pr" else emit_de)(t)
            v.wait_ge(s_pe, N_TILES)
            v.tensor_copy(out_sb, ps).then_inc(s_dve, 1)

        @block.tensor
        def _(te):
            for t in range(N_TILES):
                buf = t % 2
                te.wait_ge(s_dve, dve_idx[("e1", t)])
                for c in range(CPT):
                    col = t * CPT + c
                    sl = slice(c * CHUNK, (c + 1) * CHUNK)
                    te.matmul(ps[:, col:col + 1], t_e1[buf][:, sl], ones_bf,
                              start=True, stop=True)
                    mm = te.matmul(ps[:, COLS + col:COLS + col + 1],
                                   t_pr[buf][:, sl], ones_bf,
                                   start=True, stop=True)
                mm.then_inc(s_pe, 1)

    _check_one_wait(nc)
    return nc


def _get_program():
    if "nc" not in _CACHE:
        _CACHE["nc"] = _build_program()
    return _CACHE["nc"]


def _shard(xb):
    """bf16 [N_TOTAL] -> per-core arrays in the transposed tile layout:
    dram[t, p, f] = x[core_base + t*TILE_ELEMS + f*P + p]."""
    xt = xb.reshape(N_CORES, N_TILES, TILE_F, P).transpose(0, 1, 3, 2)
    return [np.ascontiguousarray(xt[k]).reshape(N_LOCAL) for k in range(N_CORES)]


def _run_device(yt_s, w_s, q_s, trace=False):
    from concourse.bass_utils import run_bass_kernel_spmd

    nc = _get_program()
    in_maps = [
        {"yt": yt_s[k], "w": w_s[k], "q": q_s[k]} for k in range(N_CORES)
    ]
    res = run_bass_kernel_spmd(nc, in_maps, list(range(N_CORES)), trace=trace)
    bs1 = []
    bs2 = []
    for r in res.results:
        O = np.asarray(r["o"]).reshape(P, 2 * COLS)
        bs1.append(O[:, :COLS].T.ravel())   # block b = 128*col + m
        bs2.append(O[:, COLS:].T.ravel())
    return np.concatenate(bs1), np.concatenate(bs2), res


def kernel(y_pred, y_true, weight, segment_ptr, _trace=False):
    ptr = np.asarray(segment_ptr).astype(np.int64).reshape(-1)
    n = N_TOTAL
    G = ptr.shape[0] - 1

    yp = np.ascontiguousarray(np.asarray(y_pred), dtype=np.float32)
    yt_b = np.ascontiguousarray(np.asarray(y_true), dtype=np.float32).astype(bfloat16)
    w_b = np.ascontiguousarray(np.asarray(weight), dtype=np.float32).astype(bfloat16)
    lnq_b = np.log(yp.astype(np.float64) + EPS).astype(np.float32).astype(bfloat16)

    bs1, bs2, res = _run_device(
        _shard(yt_b), _shard(w_b), _shard(lnq_b), trace=_trace)
    _CACHE["last_res"] = res

    # ---- host assembly in fp64 ----
    pre1 = np.empty(bs1.shape[0] + 1)
    pre1[0] = 0.0
    np.cumsum(bs1, dtype=np.float64, out=pre1[1:])
    pre2 = np.empty(bs2.shape[0] + 1)
    pre2[0] = 0.0
    np.cumsum(bs2, dtype=np.float64, out=pre2[1:])

    ptrc = np.clip(ptr, 0, n)
    b_idx = ptrc // BLK
    r = ptrc - b_idx * BLK
    seg_off = np.concatenate([[0], np.cumsum(r)])
    tot = int(seg_off[-1])
    part1 = np.zeros(ptrc.shape[0])
    part2 = np.zeros(ptrc.shape[0])
    if tot > 0:
        idx = np.repeat(ptrc - r, r) + (np.arange(tot) - np.repeat(seg_off[:-1], r))
        pr_h = (yt_b[idx].astype(np.float64) * w_b[idx].astype(np.float64))
        pr_h = pr_h.astype(bfloat16).astype(np.float64)
        e1_h = pr_h * (np.log(pr_h + TINY) - lnq_b[idx].astype(np.float64))
        nz = r > 0
        red_idx = np.minimum(seg_off[:-1][nz], tot - 1).astype(np.int64)
        part1[nz] = np.add.reduceat(e1_h, red_idx)
        part2[nz] = np.add.reduceat(pr_h, red_idx)

    C1 = pre1[b_idx] + part1
    C2 = pre2[b_idx] + part2
    A = np.diff(C1)
    Bg = np.diff(C2)
    S = np.maximum(Bg, EPS)
    total = np.sum((A - Bg * np.log(S)) / S) / max(G, 1)
    return np.float32(total)


# revision 5
# speedup vs baseline: 1.3052x; 1.0147x over previous
"""Graphwise KL loss (segment_reduce) on 8 trn2 NeuronCores.

Strategy (v5, bf16 + lnq input, DVE-only elementwise, full prefetch, T=8):
  Host:
    - Cast y_true/weight to bf16; precompute lnq = bf16(ln(y_pred + 1e-8)).
    - Pre-transpose each [TILE_F, 128] chunk so an SBUF column holds 128
      CONSECUTIVE elements -> PE does the block sums across partitions.
  Device (per core, 8 tiles of [128, 1024] bf16, all inputs prefetched):
    sync  : 24 HWDGE loads ordered (yt0,w0),(yt1,w1),q0,(yt2,w2),q1,...
            so pr(t) is never gated behind a q load; 2 output stores
    ACT   : warmup (Ln table load during DMA fill) ; lp(t) = Ln(pr + 1e-37)
    DVE   : pr(t) = yt*w ; d(t) = lp - lnq ; e1(t) = pr*d (bf16 2x);
            2 psum->SBUF copies
    PE    : per 128-col chunk: matmul(stationary=chunk, moving=ones[128,1])
            -> psum col = 128-element block sums; pr-sums issued right after
            pr(t), e1-sums after e1(t)  (psum cols: tile t -> e1 at
            [16t..16t+8), pr at [16t+8..16t+16))
  Host assembly (fp64): BLK=128 block sums; boundary partials from the
  bf16-cast inputs; with S_g = max(B_g, EPS):
      total = mean_g (A_g - B_g*ln(S_g)) / S_g.
"""

import numpy as np
from ml_dtypes import bfloat16

N_TOTAL = 8388608
N_CORES = 8
N_LOCAL = N_TOTAL // N_CORES      # 1048576
P = 128
TILE_F = 1024
TILE_ELEMS = P * TILE_F           # 131072
N_TILES = N_LOCAL // TILE_ELEMS   # 8
BLK = 128
CHUNK = 128
CPT = TILE_F // CHUNK             # 8
COLS = N_TILES * CPT              # 64
N_BLOCKS_LOCAL = N_LOCAL // BLK   # 8192
EPS = 1e-8
TINY = 1e-37

_CACHE = {}


def _check_one_wait(nc):
    bad = []
    for f in nc.m.functions:
        for bb in f.blocks:
            for inst in bb.instructions:
                si = inst.sync_info
                if si and si.on_wait and len(si.on_wait) > 1:
                    if "EventSem" not in type(inst).__name__:
                        bad.append((type(inst).__name__, inst.name, len(si.on_wait)))
    assert not bad, f"multi-wait instructions remain: {bad}"


def _build_program():
    import concourse.bass as bass
    import concourse.mybir as mybir

    f32 = mybir.dt.float32
    bf16 = mybir.dt.bfloat16
    Ln = mybir.ActivationFunctionType.Ln

    nc = bass.Bass()

    ct = nc.alloc_sbuf_tensor(f"const-f32-tiny", [128, 1], f32)
    mset = nc.gpsimd.memset(ct.ap(), TINY)
    nc.const_aps.aps[(f32, TINY)] = ct.ap()
    s_init = nc.alloc_semaphore("s_init")
    mset.then_inc(s_init, 1)

    yt = nc.declare_dram_parameter("yt", [N_LOCAL], bf16, isOutput=False)
    w = nc.declare_dram_parameter("w", [N_LOCAL], bf16, isOutput=False)
    q = nc.declare_dram_parameter("q", [N_LOCAL], bf16, isOutput=False)  # lnq
    o = nc.declare_dram_parameter("o", [P * 2 * COLS], f32, isOutput=True)

    yt3 = yt[:].rearrange("(t p f) -> t p f", p=P, f=TILE_F)
    w3 = w[:].rearrange("(t p f) -> t p f", p=P, f=TILE_F)
    q3 = q[:].rearrange("(t p f) -> t p f", p=P, f=TILE_F)
    o2 = o[:].rearrange("(p f) -> p f", p=P)

    def bufn(name, n, shape, dt):
        return [nc.alloc_sbuf_tensor(f"{name}{i}", shape, dt).ap() for i in range(n)]

    t_yt = bufn("t_yt", N_TILES, [P, TILE_F], bf16)
    t_w = bufn("t_w", N_TILES, [P, TILE_F], bf16)
    t_q = bufn("t_q", N_TILES, [P, TILE_F], bf16)
    t_pr = bufn("t_pr", 2, [P, TILE_F], bf16)
    t_lp = bufn("t_lp", 2, [P, TILE_F], bf16)
    t_d = bufn("t_d", 2, [P, TILE_F], bf16)
    t_e1 = bufn("t_e1", 2, [P, TILE_F], bf16)
    out_sb = nc.alloc_sbuf_tensor("out_sb", [P, 2 * COLS], f32).ap()
    warm_sb = nc.alloc_sbuf_tensor("warm_sb", [P, 1], f32).ap()

    ps = nc.alloc_psum_tensor("ps", [P, 2 * COLS], f32).ap()

    ones_bf = nc.const_aps.aps[(bf16, 1.0)]
    zero_f32 = nc.const_aps.aps[(f32, 0.0)]

    s_tw = [nc.alloc_semaphore(f"s_tw{i}") for i in range(N_TILES)]  # +32
    s_q = [nc.alloc_semaphore(f"s_q{i}") for i in range(N_TILES)]    # +16
    s_act = nc.alloc_semaphore("s_act")   # warm=1, lp(t)=t+2
    s_dve = nc.alloc_semaphore("s_dve")
    s_pe = nc.alloc_semaphore("s_pe")     # tile t e1-matmuls done -> t+1
    s_out = nc.alloc_semaphore("s_out")

    # psum column base for (tile, kind): e1 -> 16t, pr -> 16t+8
    def pcol(t, kind, c):
        return 2 * CPT * t + (0 if kind == "e1" else CPT) + c

    # DVE order: pr0 pr1 [d,e1]0 pr2 [d,e1]1 ... [d,e1]7 copy1 copy2
    dve_idx = {}
    n = 0
    order = [("pr", 0), ("pr", 1)]
    for t in range(N_TILES):
        order.append(("de", t))
        if t + 2 < N_TILES:
            order.append(("pr", t + 2))
    for kind, t in order:
        if kind == "pr":
            n += 1
            dve_idx[("pr", t)] = n
        else:
            n += 1
            dve_idx[("d", t)] = n
            n += 1
            dve_idx[("e1", t)] = n
    idx_copy1 = n + 1   # after [d,e1](N_TILES-2)... emitted near the end
    idx_copy2 = n + 2
    SPLIT = 6           # tiles 0..5 stored early (cols 0:96), 6..7 at end

    with nc.Block(no_gpsimd_drain=True) as block:

        @block.sync
        def _(s):
            # (yt0,w0),(yt1,w1),q0,(yt2,w2),q1,...,(yt7,w7),q6,q7
            s.dma_start(t_yt[0], yt3[0, :, :]).then_inc(s_tw[0], 16)
            s.dma_start(t_w[0], w3[0, :, :]).then_inc(s_tw[0], 16)
            for t in range(1, N_TILES):
                s.dma_start(t_yt[t], yt3[t, :, :]).then_inc(s_tw[t], 16)
                s.dma_start(t_w[t], w3[t, :, :]).then_inc(s_tw[t], 16)
                s.dma_start(t_q[t - 1], q3[t - 1, :, :]).then_inc(s_q[t - 1], 16)
            s.dma_start(t_q[N_TILES - 1], q3[N_TILES - 1, :, :]).then_inc(
                s_q[N_TILES - 1], 16)
            s.wait_ge(s_dve, idx_copy1)
            s.dma_start(o2[:, :2 * CPT * SPLIT],
                        out_sb[:, :2 * CPT * SPLIT]).then_inc(s_out, 16)
            s.wait_ge(s_dve, idx_copy2)
            s.dma_start(o2[:, 2 * CPT * SPLIT:],
                        out_sb[:, 2 * CPT * SPLIT:]).then_inc(s_out, 16)
            s.wait_ge(s_out, 32)

        @block.scalar
        def _(a):
            a.wait_ge(s_init, 1)
            a.activation(warm_sb, zero_f32, Ln, bias=TINY).then_inc(s_act, 1)
            for t in range(N_TILES):
                buf = t % 2
                a.wait_ge(s_dve, dve_idx[("pr", t)])
                a.activation(t_lp[buf], t_pr[buf], Ln, bias=TINY).then_inc(s_act, 1)

        @block.vector
        def _(v):
            def emit_pr(t):
                buf = t % 2
                v.wait_ge(s_tw[t], 32)
                if t >= 2:
                    v.wait_ge(s_pe, t - 1)  # PE done with pr/e1[buf] of t-2
                v.tensor_mul(t_pr[buf], t_yt[t], t_w[t]).then_inc(s_dve, 1)

            def emit_de(t):
                buf = t % 2
                v.wait_ge(s_act, t + 2)   # lp(t) done
                v.wait_ge(s_q[t], 16)     # lnq(t) loaded
                v.tensor_sub(t_d[buf], t_lp[buf], t_q[t]).then_inc(s_dve, 1)
                v.wait_ge(s_dve, dve_idx[("d", t)])  # same-engine RAW
                v.tensor_mul(t_e1[buf], t_pr[buf], t_d[buf]).then_inc(s_dve, 1)

            for kind, t in order:
                (emit_pr if kind == "pr" else emit_de)(t)
            v.wait_ge(s_pe, SPLIT)
            v.tensor_copy(out_sb[:, :2 * CPT * SPLIT],
                          ps[:, :2 * CPT * SPLIT]).then_inc(s_dve, 1)
            v.wait_ge(s_pe, N_TILES)
            v.tensor_copy(out_sb[:, 2 * CPT * SPLIT:],
                          ps[:, 2 * CPT * SPLIT:]).then_inc(s_dve, 1)

        @block.tensor
        def _(te):
            for t in range(N_TILES):
                buf = t % 2
                te.wait_ge(s_dve, dve_idx[("pr", t)])
                for c in range(CPT):
                    sl = slice(c * CHUNK, (c + 1) * CHUNK)
                    col = pcol(t, "pr", c)
                    te.matmul(ps[:, col:col + 1], t_pr[buf][:, sl], ones_bf,
                              start=True, stop=True)
                te.wait_ge(s_dve, dve_idx[("e1", t)])
                for c in range(CPT):
                    sl = slice(c * CHUNK, (c + 1) * CHUNK)
                    col = pcol(t, "e1", c)
                    mm = te.matmul(ps[:, col:col + 1], t_e1[buf][:, sl], ones_bf,
                                   start=True, stop=True)
                mm.then_inc(s_pe, 1)

    _check_one_wait(nc)
    return nc


def _get_program():
    if "nc" not in _CACHE:
        _CACHE["nc"] = _build_program()
    return _CACHE["nc"]


def _shard(xb):
    xt = xb.reshape(N_CORES, N_TILES, TILE_F, P).transpose(0, 1, 3, 2)
    return [np.ascontiguousarray(xt[k]).reshape(N_LOCAL) for k in range(N_CORES)]


def _run_device(yt_s, w_s, q_s, trace=False):
    from concourse.bass_utils import run_bass_kernel_spmd

    nc = _get_program()
    in_maps = [
        {"yt": yt_s[k], "w": w_s[k], "q": q_s[k]} for k in range(N_CORES)
    ]
    res = run_bass_kernel_spmd(nc, in_maps, list(range(N_CORES)), trace=trace)
    bs1 = []
    bs2 = []
    for r in res.results:
        O = np.asarray(r["o"]).reshape(P, N_TILES, 2, CPT)
        # block b = 128*(t*CPT + c) + m ; e1 at kind 0, pr at kind 1
        bs1.append(O[:, :, 0, :].reshape(P, COLS).T.ravel())
        bs2.append(O[:, :, 1, :].reshape(P, COLS).T.ravel())
    return np.concatenate(bs1), np.concatenate(bs2), res


def kernel(y_pred, y_true, weight, segment_ptr, _trace=False):
    ptr = np.asarray(segment_ptr).astype(np.int64).reshape(-1)
    n = N_TOTAL
    G = ptr.shape[0] - 1

    yp = np.ascontiguousarray(np.asarray(y_pred), dtype=np.float32)
    yt_b = np.ascontiguousarray(np.asarray(y_true), dtype=np.float32).astype(bfloat16)
    w_b = np.ascontiguousarray(np.asarray(weight), dtype=np.float32).astype(bfloat16)
    lnq_b = np.log(yp.astype(np.float64) + EPS).astype(np.float32).astype(bfloat16)

    bs1, bs2, res = _run_device(
        _shard(yt_b), _shard(w_b), _shard(lnq_b), trace=_trace)
    _CACHE["last_res"] = res

    # ---- host assembly in fp64 ----
    pre1 = np.empty(bs1.shape[0] + 1)
    pre1[0] = 0.0
    np.cumsum(bs1, dtype=np.float64, out=pre1[1:])
    pre2 = np.empty(bs2.shape[0] + 1)
    pre2[0] = 0.0
    np.cumsum(bs2, dtype=np.float64, out=pre2[1:])

    ptrc = np.clip(ptr, 0, n)
    b_idx = ptrc // BLK
    r = ptrc - b_idx * BLK
    seg_off = np.concatenate([[0], np.cumsum(r)])
    tot = int(seg_off[-1])
    part1 = np.zeros(ptrc.shape[0])
    part2 = np.zeros(ptrc.shape[0])
    if tot > 0:
        idx = np.repeat(ptrc - r, r) + (np.arange(tot) - np.repeat(seg_off[:-1], r))
        pr_h = (yt_b[idx].astype(np.float64) * w_b[idx].astype(np.float64))
        pr_h = pr_h.astype(bfloat16).astype(np.float64)
        e1_h = pr_h * (np.log(pr_h + TINY) - lnq_b[idx].astype(np.float64))
        nz = r > 0
        red_idx = np.minimum(seg_off[:-1][nz], tot - 1).astype(np.int64)
        part1[nz] = np.add.reduceat(e1_h, red_idx)
        part2[nz] = np.add.reduceat(pr_h, red_idx)

    C1 = pre1[b_idx] + part1
    C2 = pre2[b_idx] + part2
    A = np.diff(C1)
    Bg = np.diff(C2)
    S = np.maximum(Bg, EPS)
    total = np.sum((A - Bg * np.log(S)) / S) / max(G, 1)
    return np.float32(total)
